# revision 2
# baseline (speedup 1.0000x reference)
"""LogicLayer Trainium2 kernel (v2: dma_gather-based).

out[b, n] = sum_k softmax(w[n])_k * gate_k(a1, a2),  a1 = x[b, i1[n]], a2 = x[b, i2[n]]

All 16 gates are affine in {1, a1, a2, a1*a2}:
    out[b, n] = A0[n] + A1[n]*a1 + A2[n]*a2 + Ap[n]*a1*a2
A = softmax(w) @ C is precomputed on host ([8192, 4], tiny).

Device plan (8 NeuronCores, neuron-sharded: 1024 neurons x full 2048 batch):
  - x shipped transposed in f16 / fp8-e3m4 / u8(round(255x)) so one neuron's
    input column is one contiguous HBM row.
  - gathers via gpsimd.dma_gather (InstDMAGatherAnt): each call gathers
    N rows with ONE Q7 issue (~1 + 0.00034*N us) instead of one
    indirect_dma_start per 128 rows (~1.2 us each). 8 calls total.
  - per slot s (128 neurons x 2048 batch): u = Ap*g1 + A2, v = A1*g1 + A0,
    out = u*g2 + v.  Slots 0-3 run u,v on ACT (g1 in fp8 slots 0-1 /
    u8 slots 2-3 [u8-upconvert experiment]); slots 4-7 run u,v as 4x-packed
    DVE tensor_scalar on f16.
  - output experiments: slot 4 writes f16 then SWDGE dma_start casts f16->u8
    on the way to HBM; slot 5's final tensor_add writes a u8 tile directly
    (out255 = 253*out + 1.5 folded into the A coefficients); other slots
    write f16 via HWDGE.
"""

import numpy as np

BATCH = 2048
NIN = 8192
NNEUR = 8192
NCORES = 8
NN = NNEUR // NCORES  # neurons per core (1024)
NB = BATCH            # full batch per core
SLOTS = NN // 128     # 8
ACT_SLOTS = 4         # slots < this run their affines on the ACT engine

U8_SCALE = 253.0      # u8 output encode: u8 = 253*out + 1.5
U8_BIAS = 1.5

# gate -> (c0, c1, c2, cp) so gate_k(a1,a2) = c0 + c1*a1 + c2*a2 + cp*a1*a2
GATE_COEF = np.array(
    [
        [0, 0, 0, 0],    # FALSE
        [0, 0, 0, 1],    # AND
        [0, 1, 0, -1],   # a1 AND NOT a2
        [0, 1, 0, 0],    # a1
        [0, 0, 1, -1],   # NOT a1 AND a2
        [0, 0, 1, 0],    # a2
        [0, 1, 1, -2],   # XOR
        [0, 1, 1, -1],   # OR
        [1, -1, -1, 1],  # NOR
        [1, -1, -1, 2],  # XNOR
        [1, 0, -1, 0],   # NOT a2
        [1, 0, -1, 1],   # a1 OR NOT a2
        [1, -1, 0, 0],   # NOT a1
        [1, -1, 0, 1],   # NOT a1 OR a2
        [1, 0, 0, -1],   # NAND
        [1, 0, 0, 0],    # TRUE
    ],
    dtype=np.float32,
)  # [16, 4]

# Gather calls: (name, src, rows) where rows is a list of (slot, operand)
# in arrival order; linear gather position i -> dst[i%128, i//128, :].
# Chosen so the DVE stream (slots 4,5 first) and ACT chain (slot 0 first)
# both start as early as possible.
GATHER_CALLS = [
    ("A", "f8", [(0, 0), (1, 0)]),           # ACT slots 0,1 g1 (fp8)
    ("B1", "f16", [(4, 0), (4, 1)]),         # DVE slot 4
    ("B2", "f16", [(5, 0), (5, 1)]),         # DVE slot 5
    ("CU", "u8", [(2, 0), (3, 0)]),          # ACT slots 2,3 g1 (u8 experiment)
    ("D1", "f16", [(6, 0), (6, 1)]),         # DVE slot 6
    ("D2", "f16", [(7, 0), (7, 1)]),         # DVE slot 7
    ("E", "f16", [(0, 1), (1, 1)]),          # ACT slots 0,1 g2
    ("F", "f16", [(2, 1), (3, 1)]),          # ACT slots 2,3 g2
]

_CACHE = {}


def _build_nc():
    import concourse.bacc as bacc
    import concourse.bass as bass
    import concourse.mybir as mybir
    from concourse.tile import TileContext

    f32 = mybir.dt.float32
    f16 = mybir.dt.float16
    f8 = mybir.dt.float8e3
    u8 = mybir.dt.uint8
    i16 = mybir.dt.int16

    src_dt = {"f16": f16, "f8": f8, "u8": u8}

    nc = bacc.Bacc("TRN2")
    xt = nc.dram_tensor("xt", [NIN, NB], f16, kind="ExternalInput")
    xt8 = nc.dram_tensor("xt8", [NIN, NB], f8, kind="ExternalInput")
    xtu = nc.dram_tensor("xtu", [NIN, NB], u8, kind="ExternalInput")
    # io16[p, :]: int16 gather indices, call-major; within one call of n rows
    # the index for gather position i sits at [i%16, c0 + i//16], replicated
    # to partition groups 16..127.
    io16 = nc.dram_tensor("io16", [128, 2 * NN // 16], i16, kind="ExternalInput")
    # ac[p, c, s] = coefficient A_c for neuron (slot s, partition p)
    ac = nc.dram_tensor("ac", [128, 4, SLOTS], f32, kind="ExternalInput")
    yt = nc.dram_tensor("yt", [NN, NB], f16, kind="ExternalOutput")
    yt8c = nc.dram_tensor("yt8c", [128, NB], u8, kind="ExternalOutput")  # slot 4
    yt8d = nc.dram_tensor("yt8d", [128, NB], u8, kind="ExternalOutput")  # slot 5

    src_t = {"f16": xt, "f8": xt8, "u8": xtu}

    with TileContext(nc) as tc:
        with tc.tile_pool(name="all", bufs=1) as pool:
            it = pool.tile([128, 2 * NN // 16], i16)
            nc.sync.dma_start(it[:], io16[:])
            act = pool.tile([128, 4, SLOTS], f32)
            nc.sync.dma_start(act[:], ac[:])

            # one dest tile per gather call
            gt = {}
            for name, dt, rows in GATHER_CALLS:
                gt[name] = pool.tile([128, len(rows), NB], src_dt[dt], name=f"g_{name}")

            # (slot, operand) -> AP of its gathered rows
            gap = {}
            for name, dt, rows in GATHER_CALLS:
                for j, (s, o) in enumerate(rows):
                    gap[(s, o)] = gt[name][:, j, :]

            # issue the gathers
            col = 0
            for name, dt, rows in GATHER_CALLS:
                n = 128 * len(rows)
                ncols = n // 16
                nc.gpsimd.dma_gather(
                    out_ap=gt[name][:],
                    in_ap=src_t[dt][:],
                    idxs_ap=it[:, col:col + ncols],
                    num_idxs=n,
                    num_idxs_reg=n,
                    elem_size=NB,
                )
                col += ncols

            uv = [pool.tile([128, 2, NB], f16, name=f"uv{s}") for s in range(SLOTS)]
            ot = pool.tile([128, SLOTS, NB], f16)
            ot8 = pool.tile([128, NB], u8)  # slot 5 u8 output tile

            def slot_aps(s):
                return (gap[(s, 0)], gap[(s, 1)],
                        uv[s][:, 0, :], uv[s][:, 1, :],
                        act[:, 0, s:s + 1], act[:, 1, s:s + 1],
                        act[:, 2, s:s + 1], act[:, 3, s:s + 1])

            # ACT engine stream: serial chain u0,v0..u3,v3
            for s in range(ACT_SLOTS):
                g1, g2, u, v, A0, A1, A2, Ap = slot_aps(s)
                nc.scalar.activation(
                    u, g1, mybir.ActivationFunctionType.Identity,
                    bias=A2, scale=Ap)
                nc.scalar.activation(
                    v, g1, mybir.ActivationFunctionType.Identity,
                    bias=A0, scale=A1)

            # DVE stream: TS slots first (their g1 arrives first), each slot's
            # TT pair right after its TS pair; then the ACT slots' TTs.
            tt_order = list(range(ACT_SLOTS, SLOTS)) + list(range(ACT_SLOTS))
            for s in tt_order:
                g1, g2, u, v, A0, A1, A2, Ap = slot_aps(s)
                if s >= ACT_SLOTS:
                    nc.vector.tensor_scalar(u, g1, Ap, A2,
                                            mybir.AluOpType.mult,
                                            mybir.AluOpType.add)
                    nc.vector.tensor_scalar(v, g1, A1, A0,
                                            mybir.AluOpType.mult,
                                            mybir.AluOpType.add)
                if s == 5:
                    # u8-output experiment: final add writes the u8 tile
                    nc.vector.tensor_mul(ot[:, s, :], u, g2)
                    nc.vector.tensor_add(ot8[:], ot[:, s, :], v)
                    nc.sync.dma_start(yt8d[:, :], ot8[:])
                    continue
                nc.vector.tensor_mul(ot[:, s, :], u, g2)
                nc.vector.tensor_add(ot[:, s, :], ot[:, s, :], v)
                if s == 4:
                    # SWDGE cast-write experiment: f16 -> u8 during DMA
                    nc.gpsimd.dma_start(yt8c[:, :], ot[:, s, :])
                elif s == tt_order[-1]:
                    h = NB // 2
                    nc.sync.dma_start(yt[s * 128:(s + 1) * 128, 0:h],
                                      ot[:, s, 0:h])
                    nc.scalar.dma_start(yt[s * 128:(s + 1) * 128, h:NB],
                                        ot[:, s, h:NB])
                else:
                    nc.sync.dma_start(yt[s * 128:(s + 1) * 128, :], ot[:, s, :])

    nc.compile()
    return nc


def _prep_core_inputs(x, w, conn_indices):
    """Host-side shard/layout prep. Returns list of per-core input dicts."""
    import ml_dtypes

    xT = x.T
    xt = np.ascontiguousarray(xT.astype(np.float16))
    xt8 = np.ascontiguousarray(xT.astype(ml_dtypes.float8_e3m4)).view(np.uint8)
    xtu = np.ascontiguousarray(np.rint(xT * 255.0)).astype(np.uint8)
    # A = softmax(w) @ GATE_COEF, [NNEUR, 4] in f64 on host
    ew = np.exp(w.astype(np.float64))
    probs = ew / ew.sum(axis=1, keepdims=True)
    A = (probs @ GATE_COEF.astype(np.float64))  # [N, 4] f64: A0, A1, A2, Ap
    maps = []
    for c in range(NCORES):
        n0 = c * NN
        idx = conn_indices[n0:n0 + NN, :].reshape(SLOTS, 128, 2)  # [s, p, o]
        Ac = A[n0:n0 + NN, :].reshape(SLOTS, 128, 4).copy()       # [s, p, c]
        # per-slot coefficient folding
        for s in range(SLOTS):
            if s in (2, 3):      # u8 g1: a1 = g1/255
                Ac[s, :, 1] /= 255.0   # A1
                Ac[s, :, 3] /= 255.0   # Ap
            if s in (4, 5):      # u8 out: out255 = 253*out + 1.5
                Ac[s] *= U8_SCALE
                Ac[s, :, 0] += U8_BIAS
        # io16: call-major int16 index columns
        cols = []
        for name, dt, rows in GATHER_CALLS:
            lin = np.concatenate([idx[s, :, o] for (s, o) in rows])  # [n]
            n = lin.shape[0]
            m = lin.reshape(n // 16, 16).T.astype(np.int16)  # [16, n/16]
            cols.append(m)
        io_mat = np.concatenate(cols, axis=1)            # [16, 128]
        io16 = np.tile(io_mat, (8, 1))                   # [128, 128]
        acx = Ac.transpose(1, 2, 0).astype(np.float32)   # [p, c, s]
        maps.append({
            "xt": xt,
            "xt8": xt8,
            "xtu": xtu,
            "io16": np.ascontiguousarray(io16),
            "ac": np.ascontiguousarray(acx),
        })
    return maps


def run_cores(in_maps, trace=False):
    from concourse.bass_utils import run_bass_kernel_spmd

    if "nc" not in _CACHE:
        _CACHE["nc"] = _build_nc()
    return run_bass_kernel_spmd(
        _CACHE["nc"], in_maps, core_ids=list(range(NCORES)), trace=trace
    )


def _assemble(results):
    out = np.empty((BATCH, NNEUR), dtype=np.float32)
    for c in range(NCORES):
        n0 = c * NN
        r = results[c]
        y = r["yt"].astype(np.float32)                   # [NN, NB]
        # u8 slots: decode (u8 - 1.0)/253 (centered for truncation; see test)
        y[4 * 128:5 * 128, :] = (r["yt8c"].astype(np.float32) - 1.0) / U8_SCALE
        y[5 * 128:6 * 128, :] = (r["yt8d"].astype(np.float32) - 1.0) / U8_SCALE
        out[:, n0:n0 + NN] = y.T
    return out


def kernel(x, w, conn_indices):
    x = np.asarray(x, dtype=np.float32)
    w = np.asarray(w, dtype=np.float32)
    conn_indices = np.asarray(conn_indices)
    in_maps = _prep_core_inputs(x, w, conn_indices)
    res = run_cores(in_maps)
    return _assemble([r for r in res.results])
